# revision 2
# baseline (speedup 1.0000x reference)
"""Trainium2 Bass kernel for AdjacencyMatchingLoss (8-core SPMD).

Math: adj_score[b,e] = P[b,i_e,:] @ A @ P[b,j_e,:]  with A = (d_hw==1).
Let W[i,j] = sum_e w_e * 1[i_e=i] * 1[j_e=j]   (weighted pair histogram)
Then  total_adj = sum_ij W[i,j] * mean_b (P_b A P_b^T)[i,j]
               = (1/B) * sum_b < P_b^T W P_b , A >

Per core: edges are sharded E/8 ways (P and d_hw replicated). The pair
histogram W is built on the TensorEngine from 49 one-hot outer-product
matmuls (K = 128 edges each). The one-hot operands are built with
per-chunk DVE tensor_scalar ops: out = (iota == idx) [* w], which hit
the DVE 4x perf mode (all operands 2-byte/packed/SBUF; the per-partition
scalars are mode-exempt), so each [128,128] op is seq-overhead bound
(~70ns) rather than element bound. The weight multiply is fused into the
I-side one-hot via op0=is_equal, op1=mult.

C = sum_b P_b^T W P_b follows with 2+8 matmuls via the U = W^T P_b
factorization (all operands in natural layout, no transposes). The
kernel emits [128,2] per-partition partials of [<C,-A/8>, sum(w)]; the
host sums partials over partitions and cores and performs the final
divide (that reduction is part of unsharding the scalar output).

Host-side packing (sharding/reformatting only, no arithmetic on device
data beyond dtype rounding): P is pre-transposed to [L, B*Q] and cast
bf16 (the device used bf16 for all matmuls anyway), pair indices are
sent as f32 scalars-per-chunk, d_hw as int8. DMAs are issued from the
Pool queue (25ns seq cost vs 565ns on SP).
"""

import os
import sys

import numpy as np

for _p in ("/opt/trn_rl_repo",):
    if os.path.isdir(_p) and _p not in sys.path:
        sys.path.insert(0, _p)

B, NL, NQ, E = 8, 128, 128, 50000
NCORES = 8
ESH = E // NCORES            # 6250 edges per core
CHUNKS = (ESH + 127) // 128  # 49 chunks of <=128 edges (one per partition)
EPC = CHUNKS                 # edges per partition (49)
EPAD = 128 * EPC             # 6272
META_W = 184                 # f32 words/partition: 49 idxI | 49 idxJ | 49 w | pad
PB_W = B * NQ + 64           # bf16 words/partition: 1024 P | 64 (128 int8 d_hw)

_BUILT = None


def _emit_body(nc, sp, pp, tensors):
    import concourse.mybir as mybir

    f32 = mybir.dt.float32
    bf16 = mybir.dt.bfloat16
    i8 = mybir.dt.int8
    EQ = mybir.AluOpType.is_equal
    MUL = mybir.AluOpType.mult
    ADD = mybir.AluOpType.add
    meta_d, pb_d, o_d = tensors

    meta = sp.tile([128, META_W], f32)
    pbt = sp.tile([128, PB_W], bf16)
    iot = sp.tile([128, 128], bf16)
    OhJ = sp.tile([128, EPAD], bf16)
    OhIW = sp.tile([128, EPAD], bf16)
    Wsb = sp.tile([128, 128], bf16)
    Usb = sp.tile([128, B * NQ], bf16)
    Asc = sp.tile([128, NQ], f32)
    scr = sp.tile([128, NQ], f32)
    prt = sp.tile([128, 2], f32)

    Wps = pp.tile([128, 128], f32)
    Up0 = pp.tile([128, 512], f32)
    Up1 = pp.tile([128, 512], f32)
    Cps = pp.tile([128, 128], f32)

    # ---- loads (Pool queue: cheapest DMA issue) ----
    nc.gpsimd.dma_start(out=meta[:], in_=meta_d.ap())
    nc.gpsimd.dma_start(out=pbt[:], in_=pb_d.ap())
    nc.gpsimd.iota(
        iot[:],
        pattern=[[1, 128]],
        base=0,
        channel_multiplier=0,
        allow_small_or_imprecise_dtypes=True,
    )

    idxI = meta[:, 0:EPC]
    idxJ = meta[:, EPC : 2 * EPC]
    wT = meta[:, 2 * EPC : 3 * EPC]
    Pb = pbt[:, 0 : B * NQ]
    dsb = pbt[:, B * NQ : PB_W].bitcast(i8)   # [128,128] int8

    # sum(w) partial early (prt written once, DMA'd at the end)
    nc.vector.tensor_reduce(
        out=prt[:, 1:2], in_=wT, axis=mybir.AxisListType.X, op=ADD
    )

    # ---- one-hots + W accumulation, chunk-pipelined on PE ----
    for c in range(CHUNKS):
        sl = slice(c * 128, (c + 1) * 128)
        nc.vector.tensor_scalar(
            out=OhIW[:, sl], in0=iot[:],
            scalar1=idxI[:, c : c + 1], scalar2=wT[:, c : c + 1],
            op0=EQ, op1=MUL,
        )
        nc.vector.tensor_scalar(
            out=OhJ[:, sl], in0=iot[:],
            scalar1=idxJ[:, c : c + 1], scalar2=None, op0=EQ,
        )
        nc.tensor.matmul(
            Wps[:], lhsT=OhIW[:, sl], rhs=OhJ[:, sl],
            start=(c == 0), stop=(c == CHUNKS - 1),
        )

    # A_scaled = -(1/8) * (d_hw == 1); folds sign + batch-mean
    nc.gpsimd.tensor_scalar(
        out=Asc[:], in0=dsb, scalar1=1, scalar2=-0.125, op0=EQ, op1=MUL
    )

    # ---- U = W^T P_b ; C = sum_b U_b^T(P-contract) ----
    nc.scalar.copy(out=Wsb[:], in_=Wps[:])
    nc.tensor.matmul(Up0[:], lhsT=Wsb[:], rhs=Pb[:, 0:512], start=True, stop=True)
    nc.tensor.matmul(Up1[:], lhsT=Wsb[:], rhs=Pb[:, 512:1024], start=True, stop=True)
    nc.scalar.copy(out=Usb[:, 0:512], in_=Up0[:])
    nc.vector.tensor_copy(out=Usb[:, 512:1024], in_=Up1[:])
    for b in range(B):
        sl = slice(b * 128, (b + 1) * 128)
        nc.tensor.matmul(
            Cps[:], lhsT=Usb[:, sl], rhs=Pb[:, sl],
            start=(b == 0), stop=(b == B - 1),
        )

    # ---- partials: [ <C, -A/8> , sum(w) ] ----
    nc.vector.tensor_tensor(out=scr[:], in0=Cps[:], in1=Asc[:], op=MUL)
    nc.vector.tensor_reduce(
        out=prt[:, 0:1], in_=scr[:], axis=mybir.AxisListType.X, op=ADD
    )
    # partition + cross-core reduction of the [128,2] partials on host
    nc.gpsimd.dma_start(out=o_d.ap(), in_=prt[:])


def _build(reps=1):
    import concourse.bacc as bacc
    import concourse.mybir as mybir
    import concourse.tile as tile

    f32 = mybir.dt.float32
    bf16 = mybir.dt.bfloat16

    nc = bacc.Bacc("TRN2", target_bir_lowering=False, debug=False, num_devices=NCORES)

    meta_d = nc.dram_tensor("meta_in", [128, META_W], f32, kind="ExternalInput")
    pb_d = nc.dram_tensor("pb_in", [128, PB_W], bf16, kind="ExternalInput")
    o_d = nc.dram_tensor("out", [128, 2], f32, kind="ExternalOutput")

    with tile.TileContext(nc) as tc:
        with (
            tc.tile_pool(name="sbuf", bufs=1) as sp,
            tc.tile_pool(name="psum", bufs=1, space="PSUM") as pp,
        ):
            for _ in range(reps):
                _emit_body(nc, sp, pp, (meta_d, pb_d, o_d))

    nc.compile()
    return nc


def _get_built():
    global _BUILT
    if _BUILT is None:
        _BUILT = _build()
    return _BUILT


def _shard_inputs(P, d_hw, circuit_edge_pairs, circuit_edge_weights):
    P = np.asarray(P, dtype=np.float32)
    d_hw = np.asarray(d_hw, dtype=np.int32)
    pairs = np.asarray(circuit_edge_pairs).astype(np.int64, copy=False)
    w = np.asarray(circuit_edge_weights, dtype=np.float32)

    # per-core edge shard, padded to 128*49 with idx=0 / w=0
    pairs_pad = np.zeros((NCORES, EPAD, 2), dtype=np.float32)
    w_pad = np.zeros((NCORES, EPAD), dtype=np.float32)
    pairs_pad[:, :ESH] = pairs.reshape(NCORES, ESH, 2)
    w_pad[:, :ESH] = w.reshape(NCORES, ESH)
    # edge (p, c) = local edge p*EPC + c
    idxI = pairs_pad[:, :, 0].reshape(NCORES, 128, EPC)
    idxJ = pairs_pad[:, :, 1].reshape(NCORES, 128, EPC)
    wPC = w_pad.reshape(NCORES, 128, EPC)

    meta = np.zeros((NCORES, 128, META_W), dtype=np.float32)
    meta[:, :, 0:EPC] = idxI
    meta[:, :, EPC : 2 * EPC] = idxJ
    meta[:, :, 2 * EPC : 3 * EPC] = wPC

    # P transposed to [L, B*Q] bf16 + d_hw (int8) in the same tensor
    import ml_dtypes

    Pt = (
        P.transpose(1, 0, 2)
        .reshape(NL, B * NQ)
        .astype(ml_dtypes.bfloat16)
    )
    pb = np.zeros((128, PB_W), dtype=ml_dtypes.bfloat16)
    pb[:, 0 : B * NQ] = Pt
    pb[:, B * NQ : PB_W] = (
        d_hw.astype(np.int8).reshape(128, 128).view(np.int16).view(ml_dtypes.bfloat16)
    )

    return [
        {"meta_in": np.ascontiguousarray(meta[i]), "pb_in": pb}
        for i in range(NCORES)
    ]


def _combine(results):
    parts = np.stack([np.asarray(results[i]["out"]) for i in range(NCORES)])
    numer = float(parts[:, :, 0].astype(np.float64).sum())
    wsum = float(parts[:, :, 1].astype(np.float64).sum())
    return np.float32(numer / max(wsum, 1e-8))


def make_runner(nc, n_cores=NCORES):
    """jit-once mirror of bass2jax.run_bass_via_pjrt's multi-core branch so
    repeated kernel() calls reuse the compiled NEFF."""
    import jax
    import concourse.mybir as mybir
    from concourse.bass2jax import (
        Mesh,
        PartitionSpec,
        _bass_exec_p,
        install_neuronx_cc_hook,
        partition_id_tensor,
        shard_map,
    )

    install_neuronx_cc_hook()
    partition_name = nc.partition_id_tensor.name if nc.partition_id_tensor else None

    in_names, out_names, out_avals, zero_outs = [], [], [], []
    for alloc in nc.m.functions[0].allocations:
        if not isinstance(alloc, mybir.MemoryLocationSet):
            continue
        name = alloc.memorylocations[0].name
        if alloc.kind == "ExternalInput":
            if name != partition_name:
                in_names.append(name)
        elif alloc.kind == "ExternalOutput":
            shape = tuple(alloc.tensor_shape)
            dtype = mybir.dt.np(alloc.dtype)
            out_names.append(name)
            out_avals.append(jax.core.ShapedArray(shape, dtype))
            zero_outs.append(np.zeros(shape, dtype))
    n_params = len(in_names)
    n_outs = len(out_avals)
    all_names = in_names + out_names
    if partition_name is not None:
        all_names = all_names + [partition_name]
    donate = tuple(range(n_params, n_params + n_outs))

    def _body(*args):
        operands = list(args)
        if partition_name is not None:
            operands.append(partition_id_tensor())
        outs = _bass_exec_p.bind(
            *operands,
            out_avals=tuple(out_avals),
            in_names=tuple(all_names),
            out_names=tuple(out_names),
            lowering_input_output_aliases=(),
            sim_require_finite=True,
            sim_require_nnan=True,
            nc=nc,
        )
        return tuple(outs)

    devices = jax.devices()[:n_cores]
    mesh = Mesh(np.asarray(devices), ("core",))
    sharded = jax.jit(
        shard_map(
            _body,
            mesh=mesh,
            in_specs=(PartitionSpec("core"),) * (n_params + n_outs),
            out_specs=(PartitionSpec("core"),) * n_outs,
            check_rep=False,
        ),
        donate_argnums=donate,
        keep_unused=True,
    )

    def prep(in_maps):
        concat_in = [
            np.concatenate([np.asarray(m[name]) for m in in_maps], axis=0)
            for name in in_names
        ]
        return [jax.device_put(a) for a in concat_in]

    def run_dev(dev_in):
        concat_zeros = [
            np.zeros((n_cores * z.shape[0], *z.shape[1:]), z.dtype)
            for z in zero_outs
        ]
        out_arrs = sharded(*dev_in, *concat_zeros)
        out_arrs = [np.asarray(a) for a in out_arrs]
        return [
            {
                name: out_arrs[i].reshape(n_cores, *out_avals[i].shape)[c]
                for i, name in enumerate(out_names)
            }
            for c in range(n_cores)
        ]

    def run(in_maps):
        return run_dev(prep(in_maps))

    run.prep = prep
    run.run_dev = run_dev
    return run


_RUNNER = None


def kernel(P, d_hw, circuit_edge_pairs, circuit_edge_weights, _want_results=False):
    global _RUNNER
    in_maps = _shard_inputs(P, d_hw, circuit_edge_pairs, circuit_edge_weights)
    try:
        if _RUNNER is None:
            _RUNNER = make_runner(_get_built())
        results = _RUNNER(in_maps)
        res = None
    except Exception:
        if _want_results:
            raise
        # fallback: the stock SPMD runner (recompiles per call, but robust)
        from concourse.bass_utils import run_bass_kernel_spmd

        res = run_bass_kernel_spmd(
            _get_built(), in_maps, core_ids=list(range(NCORES))
        )
        results = res.results
    out = _combine(results)
    if _want_results:
        return out, res
    return out


# revision 11
# speedup vs baseline: 1.8342x; 1.8342x over previous
"""Trainium2 Bass kernel for AdjacencyMatchingLoss (8-core SPMD).

Math: adj_score[b,e] = P[b,i_e,:] @ A @ P[b,j_e,:]  with A = (d_hw==1).
Let W[i,j] = sum_e w_e * 1[i_e=i] * 1[j_e=j]   (weighted pair histogram)
Then  total_adj = sum_ij W[i,j] * mean_b (P_b A P_b^T)[i,j]
               = (1/B) * sum_b < P_b^T W P_b , A >

Per core: edges are sharded E/8 ways (P and d_hw replicated). The pair
histogram W is built on the TensorEngine from 49 one-hot outer-product
matmuls (K = 128 edges each). The one-hot operands are built with
per-chunk DVE tensor_scalar ops: out = (iota == idx) [* w], which hit
the DVE 4x perf mode (all operands 2-byte/packed/SBUF; the per-partition
scalars are mode-exempt), so each [128,128] op is seq-overhead bound
(~70ns) rather than element bound. The weight multiply is fused into the
I-side one-hot via op0=is_equal, op1=mult.

C = sum_b P_b^T W P_b follows with 2+8 matmuls via the U = W^T P_b
factorization (all operands in natural layout, no transposes). The
kernel emits [128,2] per-partition partials of [<C,-A/8>, sum(w)]; the
host sums partials over partitions and cores and performs the final
divide (that reduction is part of unsharding the scalar output).

Host-side packing (sharding/reformatting only, no arithmetic on device
data beyond dtype rounding): P is pre-transposed to [L, B*Q] and cast
bf16 (the device used bf16 for all matmuls anyway), pair indices are
sent as f32 scalars-per-chunk, d_hw as int8. DMAs are issued from the
Pool queue (25ns seq cost vs 565ns on SP).
"""

import os
import sys

import numpy as np

for _p in ("/opt/trn_rl_repo",):
    if os.path.isdir(_p) and _p not in sys.path:
        sys.path.insert(0, _p)

B, NL, NQ, E = 8, 128, 128, 50000
NCORES = 8
ESH = E // NCORES            # 6250 edges per core
CHUNKS = (ESH + 127) // 128  # 49 chunks of <=128 edges (one per partition)
EPC = CHUNKS                 # edges per partition (49)
EPAD = 128 * EPC             # 6272
META_W = 200                 # f32 words/partition: idxI | idxJ | w | -w | pad
PB_W = B * NQ + 64           # bf16 words/partition: 1024 P^T | 64 d_hw i8

# engine quotas for the 49 one-hot chunks: (DVE, Pool, ACT)
QUOTA = (33, 12, 4)


def _assign(n, quota):
    """Bresenham-interleaved engine assignment for n chunks."""
    order = []
    filled = [0, 0, 0]
    for i in range(n):
        # pick engine with lowest filled/quota ratio
        best = min(
            (e for e in range(3) if quota[e] > 0),
            key=lambda e: (filled[e] + 1) / quota[e],
        )
        filled[best] += 1
        order.append(best)
    return order

_BUILT = None


def _emit_body(nc, sp, pp, tensors):
    import concourse.mybir as mybir

    f32 = mybir.dt.float32
    bf16 = mybir.dt.bfloat16
    i8 = mybir.dt.int8
    EQ = mybir.AluOpType.is_equal
    MUL = mybir.AluOpType.mult
    ADD = mybir.AluOpType.add
    ABS = mybir.ActivationFunctionType.Abs
    RELU = mybir.ActivationFunctionType.Relu
    meta_d, pb_d, o_d = tensors

    meta = sp.tile([128, META_W], f32)
    pbt = sp.tile([128, PB_W], bf16)
    iot = sp.tile([128, 128], bf16)
    OhJ = sp.tile([128, EPAD], bf16)
    OhIW = sp.tile([128, EPAD], bf16)
    Asc = sp.tile([128, NQ], bf16)
    Ysb = sp.tile([128, B * NQ], bf16)
    Ssb = sp.tile([128, NQ], f32)
    scr = sp.tile([128, NQ], f32)
    prt = sp.tile([128, 2], f32)
    tmpA = [sp.tile([128, 128], bf16, name=f"tmpact{k}") for k in range(3)]

    Wps = pp.tile([128, 128], f32)
    Yp0 = pp.tile([128, 512], f32)
    Yp1 = pp.tile([128, 512], f32)
    Sps = pp.tile([128, 128], f32)

    # ---- loads (Pool queue: cheapest DMA issue) ----
    nc.gpsimd.dma_start(out=meta[:], in_=meta_d.ap())
    nc.gpsimd.dma_start(out=pbt[:], in_=pb_d.ap())
    nc.gpsimd.iota(
        iot[:],
        pattern=[[1, 128]],
        base=0,
        channel_multiplier=0,
        allow_small_or_imprecise_dtypes=True,
    )

    idxI = meta[:, 0:EPC]
    idxJ = meta[:, EPC : 2 * EPC]
    wT = meta[:, 2 * EPC : 3 * EPC]
    wNeg = meta[:, 3 * EPC : 4 * EPC]
    PT = pbt[:, 0 : B * NQ]                          # [q, (b,l)] = P[b,l,q]
    dsb = pbt[:, B * NQ : PB_W].bitcast(i8)          # [128,128] int8

    # sum(w) partial early (prt written once, DMA'd at the end)
    nc.vector.tensor_reduce(
        out=prt[:, 1:2], in_=wT, axis=mybir.AxisListType.X, op=ADD
    )
    # A_scaled = -(1/8) * (d_hw == 1); folds sign + batch-mean. Exact in bf16.
    nc.gpsimd.tensor_scalar(
        out=Asc[:], in0=dsb, scalar1=1, scalar2=-0.125, op0=EQ, op1=MUL
    )

    # ---- S = sum_b P_b Asc P_b^T (edge-independent; runs in the window) ----
    # Y[q',(b,l)] = sum_q Asc[q,q'] P[b,l,q] ; S[l,l'] = sum_{b,q'} Y PT
    def emit_s():
        nc.tensor.matmul(Yp0[:], lhsT=Asc[:], rhs=PT[:, 0:512], start=True, stop=True)
        nc.tensor.matmul(Yp1[:], lhsT=Asc[:], rhs=PT[:, 512:1024], start=True, stop=True)
        nc.gpsimd.tensor_copy(out=Ysb[:, 0:512], in_=Yp0[:])
        nc.scalar.copy(out=Ysb[:, 512:1024], in_=Yp1[:])
        for b in range(B):
            sl = slice(b * 128, (b + 1) * 128)
            nc.tensor.matmul(
                Sps[:], lhsT=Ysb[:, sl], rhs=PT[:, sl],
                start=(b == 0), stop=(b == B - 1),
            )
        nc.gpsimd.tensor_copy(out=Ssb[:], in_=Sps[:])

    # ---- one-hots on 3 engines + W accumulation, chunk-pipelined on PE ----
    def emit_chunk(c, eng):
        sl = slice(c * 128, (c + 1) * 128)
        if eng == 2:  # ACT: relu(1-|i-idx|) / relu(w - w|i-idx|) trick
            tJ, tI = tmpA[c % 2], tmpA[(c % 2) ^ 1]
            nc.scalar.activation(
                out=tJ[:], in_=iot[:], func=ABS,
                bias=idxJ[:, c : c + 1], scale=-1.0,
            )
            nc.scalar.activation(
                out=OhJ[:, sl], in_=tJ[:], func=RELU, bias=1.0, scale=-1.0
            )
            nc.scalar.activation(
                out=tI[:], in_=iot[:], func=ABS,
                bias=idxI[:, c : c + 1], scale=-1.0,
            )
            nc.scalar.activation(
                out=OhIW[:, sl], in_=tI[:], func=RELU,
                bias=wT[:, c : c + 1], scale=wNeg[:, c : c + 1],
            )
        else:
            e = nc.vector if eng == 0 else nc.gpsimd
            e.tensor_scalar(
                out=OhIW[:, sl], in0=iot[:],
                scalar1=idxI[:, c : c + 1], scalar2=wT[:, c : c + 1],
                op0=EQ, op1=MUL,
            )
            e.tensor_scalar(
                out=OhJ[:, sl], in0=iot[:],
                scalar1=idxJ[:, c : c + 1], scalar2=None, op0=EQ,
            )
        nc.tensor.matmul(
            Wps[:], lhsT=OhIW[:, sl], rhs=OhJ[:, sl],
            start=(c == 0), stop=(c == CHUNKS - 1),
        )

    assign = _assign(CHUNKS, QUOTA)
    for c in range(CHUNKS):
        emit_chunk(c, assign[c])
        if c == 1:
            emit_s()

    # ---- partials: [ <W, S> , sum(w) ] ----
    nc.vector.tensor_tensor(out=scr[:], in0=Wps[:], in1=Ssb[:], op=MUL)
    nc.vector.tensor_reduce(
        out=prt[:, 0:1], in_=scr[:], axis=mybir.AxisListType.X, op=ADD
    )
    # partition + cross-core reduction of the [128,2] partials on host
    nc.gpsimd.dma_start(out=o_d.ap(), in_=prt[:])


def _build(reps=1):
    import concourse.bacc as bacc
    import concourse.mybir as mybir
    import concourse.tile as tile

    f32 = mybir.dt.float32
    bf16 = mybir.dt.bfloat16

    nc = bacc.Bacc("TRN2", target_bir_lowering=False, debug=False, num_devices=NCORES)

    meta_d = nc.dram_tensor("meta_in", [128, META_W], f32, kind="ExternalInput")
    pb_d = nc.dram_tensor("pb_in", [128, PB_W], bf16, kind="ExternalInput")
    o_d = nc.dram_tensor("out", [128, 2], f32, kind="ExternalOutput")

    with tile.TileContext(nc) as tc:
        with (
            tc.tile_pool(name="sbuf", bufs=1) as sp,
            tc.tile_pool(name="psum", bufs=1, space="PSUM") as pp,
        ):
            for _ in range(reps):
                _emit_body(nc, sp, pp, (meta_d, pb_d, o_d))

    nc.compile()
    return nc


def _get_built():
    global _BUILT
    if _BUILT is None:
        _BUILT = _build()
    return _BUILT


def _shard_inputs(P, d_hw, circuit_edge_pairs, circuit_edge_weights):
    P = np.asarray(P, dtype=np.float32)
    d_hw = np.asarray(d_hw, dtype=np.int32)
    pairs = np.asarray(circuit_edge_pairs).astype(np.int64, copy=False)
    w = np.asarray(circuit_edge_weights, dtype=np.float32)

    # per-core edge shard, padded to 128*49 with idx=0 / w=0
    pairs_pad = np.zeros((NCORES, EPAD, 2), dtype=np.float32)
    w_pad = np.zeros((NCORES, EPAD), dtype=np.float32)
    pairs_pad[:, :ESH] = pairs.reshape(NCORES, ESH, 2)
    w_pad[:, :ESH] = w.reshape(NCORES, ESH)
    # edge (p, c) = local edge p*EPC + c
    idxI = pairs_pad[:, :, 0].reshape(NCORES, 128, EPC)
    idxJ = pairs_pad[:, :, 1].reshape(NCORES, 128, EPC)
    wPC = w_pad.reshape(NCORES, 128, EPC)

    meta = np.zeros((NCORES, 128, META_W), dtype=np.float32)
    meta[:, :, 0:EPC] = idxI
    meta[:, :, EPC : 2 * EPC] = idxJ
    meta[:, :, 2 * EPC : 3 * EPC] = wPC
    meta[:, :, 3 * EPC : 4 * EPC] = -wPC

    # P transposed to [Q, B*L] bf16 + d_hw (int8) in the same tensor
    import ml_dtypes

    Pt = (
        P.transpose(2, 0, 1)
        .reshape(NQ, B * NL)
        .astype(ml_dtypes.bfloat16)
    )
    pb = np.zeros((128, PB_W), dtype=ml_dtypes.bfloat16)
    pb[:, 0 : B * NQ] = Pt
    pb[:, B * NQ : PB_W] = (
        d_hw.astype(np.int8).reshape(128, 128).view(np.int16).view(ml_dtypes.bfloat16)
    )

    return [
        {"meta_in": np.ascontiguousarray(meta[i]), "pb_in": pb}
        for i in range(NCORES)
    ]


def _combine(results):
    parts = np.stack([np.asarray(results[i]["out"]) for i in range(NCORES)])
    numer = float(parts[:, :, 0].astype(np.float64).sum())
    wsum = float(parts[:, :, 1].astype(np.float64).sum())
    return np.float32(numer / max(wsum, 1e-8))


def make_runner(nc, n_cores=NCORES):
    """jit-once mirror of bass2jax.run_bass_via_pjrt's multi-core branch so
    repeated kernel() calls reuse the compiled NEFF."""
    import jax
    import concourse.mybir as mybir
    from concourse.bass2jax import (
        Mesh,
        PartitionSpec,
        _bass_exec_p,
        install_neuronx_cc_hook,
        partition_id_tensor,
        shard_map,
    )

    install_neuronx_cc_hook()
    partition_name = nc.partition_id_tensor.name if nc.partition_id_tensor else None

    in_names, out_names, out_avals, zero_outs = [], [], [], []
    for alloc in nc.m.functions[0].allocations:
        if not isinstance(alloc, mybir.MemoryLocationSet):
            continue
        name = alloc.memorylocations[0].name
        if alloc.kind == "ExternalInput":
            if name != partition_name:
                in_names.append(name)
        elif alloc.kind == "ExternalOutput":
            shape = tuple(alloc.tensor_shape)
            dtype = mybir.dt.np(alloc.dtype)
            out_names.append(name)
            out_avals.append(jax.core.ShapedArray(shape, dtype))
            zero_outs.append(np.zeros(shape, dtype))
    n_params = len(in_names)
    n_outs = len(out_avals)
    all_names = in_names + out_names
    if partition_name is not None:
        all_names = all_names + [partition_name]
    donate = tuple(range(n_params, n_params + n_outs))

    def _body(*args):
        operands = list(args)
        if partition_name is not None:
            operands.append(partition_id_tensor())
        outs = _bass_exec_p.bind(
            *operands,
            out_avals=tuple(out_avals),
            in_names=tuple(all_names),
            out_names=tuple(out_names),
            lowering_input_output_aliases=(),
            sim_require_finite=True,
            sim_require_nnan=True,
            nc=nc,
        )
        return tuple(outs)

    devices = jax.devices()[:n_cores]
    mesh = Mesh(np.asarray(devices), ("core",))
    sharded = jax.jit(
        shard_map(
            _body,
            mesh=mesh,
            in_specs=(PartitionSpec("core"),) * (n_params + n_outs),
            out_specs=(PartitionSpec("core"),) * n_outs,
            check_rep=False,
        ),
        donate_argnums=donate,
        keep_unused=True,
    )

    def prep(in_maps):
        concat_in = [
            np.concatenate([np.asarray(m[name]) for m in in_maps], axis=0)
            for name in in_names
        ]
        return [jax.device_put(a) for a in concat_in]

    def run_dev(dev_in):
        concat_zeros = [
            np.zeros((n_cores * z.shape[0], *z.shape[1:]), z.dtype)
            for z in zero_outs
        ]
        out_arrs = sharded(*dev_in, *concat_zeros)
        out_arrs = [np.asarray(a) for a in out_arrs]
        return [
            {
                name: out_arrs[i].reshape(n_cores, *out_avals[i].shape)[c]
                for i, name in enumerate(out_names)
            }
            for c in range(n_cores)
        ]

    def run(in_maps):
        return run_dev(prep(in_maps))

    run.prep = prep
    run.run_dev = run_dev
    return run


_RUNNER = None


def kernel(P, d_hw, circuit_edge_pairs, circuit_edge_weights, _want_results=False):
    global _RUNNER
    in_maps = _shard_inputs(P, d_hw, circuit_edge_pairs, circuit_edge_weights)
    try:
        if _RUNNER is None:
            _RUNNER = make_runner(_get_built())
        results = _RUNNER(in_maps)
        res = None
    except Exception:
        if _want_results:
            raise
        # fallback: the stock SPMD runner (recompiles per call, but robust)
        from concourse.bass_utils import run_bass_kernel_spmd

        res = run_bass_kernel_spmd(
            _get_built(), in_maps, core_ids=list(range(NCORES))
        )
        results = res.results
    out = _combine(results)
    if _want_results:
        return out, res
    return out
